# revision 25
# baseline (speedup 1.0000x reference)
"""Trainium2 Bass kernel for ChunkMessagePassing (gnn_message_passing).

Problem: B=2, N=4096, D=512, 3 rounds of causal windowed (W=8) message
passing. Per round:
    A = h @ w1_top + b1 ; Bv = h @ w1_bot          (first MLP layer, factored)
    S[i] = sum_{k=0..8, valid} gelu(A[i] + Bv[i-k])
    U = h @ u1_top + S @ Wf + ub1'                 (agg matmul fused away:
        Wf = (w2/9) @ u1_bot, ub1' = ub1 + b2 @ u1_bot, host-side)
    new_h = h + gelu(U) @ u2 + ub2 ; h = LN(new_h)

Sharding: 8 cores = B(2) x N-quarters(4). Each core gets 1024 tokens plus a
24-token left halo (3 rounds x window 8), computed redundantly. Zero
cross-core communication. Cores at a sequence start get a zero pad instead
of a halo plus a data-driven edge fixup (invalid window taps excluded,
window count < 9) so all 8 cores run one SPMD program.

Layout: D on partitions (4 tiles of 128), tokens on the free axis.
Matmuls in fp32r (1 cyc/col at free>=256, ~1e-4 rel err). Window stage in
bf16 for DVE 2x mode; the 9 shifted adds are 2 DVE instructions per d-tile
using overlapping-window 3D access patterns (even taps from Bv, odd taps
from a +1-shifted copy Bvo so every window stays 4B-aligned; Bvo is built
by SBUF->SBUF DMA, off the compute engines). LN stats via ones-matmul over
partitions (pre-broadcast). LN gamma/beta are folded into the weights/
biases host-side (h buffers carry y = (new_h - mu)*rstd; the residual path
uses a diag(gamma) matmul; round-0 input is pre-transformed on the host),
so the per-round LN apply is just two tensor ops; halo re-zeroing rides the
chunk-0 rstd vector.

Cross-round software pipelining: engines execute their queues strictly
in emission order, so the last chunk's LN-stats matmuls (which wait on
that chunk's GpSimd x^2) must not be emitted ahead of the next round's
ready stage-1 matmuls. Rounds 0/1 therefore defer the last chunk's
stats + rsqrt + apply into the next round, right after its first stage-1
block; the second-to-last chunk's apply is emitted after the last chunk's
x^2. The rsqrt for the remaining chunks runs as one ACT call (bounding
gelu<->rsqrt table swaps at 2/round); the last round runs rsqrt per chunk
so the final LN + output DMA pipeline. Round 0 splits its first chunk in
two so the serial ACT gelu block at cold-start is shorter. Chunk-0 applies
run on Vector (critical path); later chunks' applies run wholly on GpSimd.
"""

import numpy as np
import ml_dtypes

import concourse.bacc as bacc
import concourse.mybir as mybir
from concourse.ap import AP
from concourse.tile import TileContext
from concourse.bass_utils import run_bass_kernel_spmd

f32 = mybir.dt.float32
f32r = mybir.dt.float32r
bf16 = mybir.dt.bfloat16
AF = mybir.ActivationFunctionType
ALU = mybir.AluOpType

B, N, D = 2, 4096, 512
N_ROUNDS = 3
W = 8
W9 = W + 1
NCORES = 8
NLOC = N // 4            # tokens owned per core
HALO = N_ROUNDS * W      # 24
T = NLOC + HALO          # 1048 local tokens incl. halo
DT = 4                   # number of 128-partition d tiles
P = 128
MARG = 8                 # zero margin on the left of Bv buffers
CN = 352                 # max chunk width (>=256 keeps fp32r at 1 cyc/row)
CHUNKS = [(0, 352), (352, 352), (704, 344)]
R0CHUNKS = [(0, 176), (176, 176), (352, 352), (704, 344)]
EPS = 1e-5


def _win_ap(base_ap, nk, kstride):
    """Overlapping-window AP: [P, nk, cn] over a [P, cn] slice, window
    stride `kstride` elements (innermost stays step-1 for DVE 2x mode)."""
    return AP(tensor=base_ap.tensor, offset=base_ap.offset,
              ap=[base_ap.ap[0], [kstride, nk], [1, base_ap.ap[1][1]]])


def build_nc():
    nc = bacc.Bacc("TRN2")

    # ---- DRAM I/O (per-core data supplied via in_maps) ----
    h_in = nc.dram_tensor("h_in", [DT, P, T], f32r, kind="ExternalInput")
    w1t_d = nc.dram_tensor("w1t", [DT, P, D], f32r, kind="ExternalInput")
    w1b_d = nc.dram_tensor("w1b", [DT, P, D], f32r, kind="ExternalInput")
    u1t_d = nc.dram_tensor("u1t", [DT, P, D], f32r, kind="ExternalInput")
    wf_d = nc.dram_tensor("wf", [DT, P, D], bf16, kind="ExternalInput")
    u2_d = nc.dram_tensor("u2", [DT, P, D], bf16, kind="ExternalInput")
    b1_d = nc.dram_tensor("b1", [P, DT], f32, kind="ExternalInput")
    ub1_d = nc.dram_tensor("ub1", [P, DT], f32, kind="ExternalInput")
    ub2_d = nc.dram_tensor("ub2", [P, DT], f32, kind="ExternalInput")
    lng_d = nc.dram_tensor("lng", [P, DT], f32, kind="ExternalInput")
    lnb_d = nc.dram_tensor("lnb", [P, DT], f32, kind="ExternalInput")
    idg_d = nc.dram_tensor("idg", [DT, P, P], f32r, kind="ExternalInput")
    ea_d = nc.dram_tensor("edge_a", [P, W], bf16, kind="ExternalInput")
    es_d = nc.dram_tensor("edge_s", [P, W], bf16, kind="ExternalInput")
    hm_d = nc.dram_tensor("hmask", [P, HALO], f32, kind="ExternalInput")
    out_d = nc.dram_tensor("out", [DT, P, NLOC], f32, kind="ExternalOutput")

    with nc.allow_low_precision("bf16/f32r compute validated against reference"), \
            TileContext(nc) as tc:
        with (
            tc.tile_pool(name="const", bufs=1) as cp,
            tc.tile_pool(name="acts", bufs=1) as ap,
            tc.tile_pool(name="wsc", bufs=3) as wp,
            tc.tile_pool(name="psab", bufs=3, space="PSUM") as psab,
            tc.tile_pool(name="ps", bufs=3, space="PSUM") as ps,
            tc.tile_pool(name="psr", bufs=2, space="PSUM") as psr,
        ):
            # ---- constants into SBUF ----
            w1t = cp.tile([P, DT * D], f32r, tag="w1t")
            w1b = cp.tile([P, DT * D], f32r, tag="w1b")
            u1t = cp.tile([P, DT * D], f32r, tag="u1t")
            wf = cp.tile([P, DT * D], bf16, tag="wf")
            u2 = cp.tile([P, DT * D], bf16, tag="u2")
            idg = cp.tile([P, DT * P], f32r, tag="idg")
            # w1t first (gates stage 1); w1b on its own queue in parallel
            nc.sync.dma_start(
                out=w1t[:].rearrange("p (k d) -> p k d", k=DT),
                in_=w1t_d.rearrange("k p d -> p k d"))
            nc.scalar.dma_start(
                out=w1b[:].rearrange("p (k d) -> p k d", k=DT),
                in_=w1b_d.rearrange("k p d -> p k d"))
            ones_fe = cp.tile([P, 1], f32, tag="ones_fe")
            gdume = cp.tile([P, 1], f32, tag="gdume")
            nc.vector.memset(ones_fe[:], 1.0)
            nc.scalar.activation(gdume[:], ones_fe[:], AF.Gelu)
            b1 = cp.tile([P, DT], f32, tag="b1")
            ub1 = cp.tile([P, DT], f32, tag="ub1")
            ub2 = cp.tile([P, DT], f32, tag="ub2")
            lng = cp.tile([P, DT], f32, tag="lng")
            lnb = cp.tile([P, DT], f32, tag="lnb")
            edge_a = cp.tile([P, W], bf16, tag="edge_a")
            edge_s = cp.tile([P, W], bf16, tag="edge_s")
            hmask = cp.tile([P, HALO], f32, tag="hmask")
            for t_sb, t_d in ((b1, b1_d), (ub1, ub1_d), (ub2, ub2_d),
                              (lng, lng_d), (lnb, lnb_d), (edge_a, ea_d),
                              (edge_s, es_d), (hmask, hm_d)):
                nc.scalar.dma_start(out=t_sb[:], in_=t_d[:])

            ones_sq = cp.tile([P, P], f32r, tag="ones_sq")     # stats lhsT (bcast out)
            ones_f = cp.tile([P, 1], f32, tag="ones_f")
            nc.vector.memset(ones_f[:], 1.0)
            nc.vector.tensor_copy(ones_sq[:], ones_f[:].to_broadcast([P, P]))
            czero = cp.tile([P, 1], f32, tag="czero")
            ceps = cp.tile([P, 1], f32, tag="ceps")
            nc.vector.memset(czero[:], 0.0)
            nc.vector.memset(ceps[:], EPS)
            nc.const_aps.aps[(f32, 0.0)] = czero[:]
            nc.const_aps.aps[(f32, EPS)] = ceps[:]

            # ---- activations (persistent, reused across rounds) ----
            h0 = ap.tile([P, DT * T], f32r, tag="h0")
            h1 = ap.tile([P, DT * T], f32r, tag="h1")
            A = ap.tile([P, DT * T], bf16, tag="A")
            BVW = MARG + T + 2
            Bv = ap.tile([P, DT * BVW], bf16, tag="Bv")
            Bvo = ap.tile([P, DT * BVW], bf16, tag="Bvo")
            S = ap.tile([P, DT * T], bf16, tag="S")
            x2 = ap.tile([P, DT * T], f32r, tag="x2")
            nmub = ap.tile([P, T], f32, tag="nmub")
            varb = ap.tile([P, T], f32, tag="varb")
            rstb = ap.tile([P, T], f32, tag="rstb")

            ga8 = ap.tile([P, W], bf16, tag="ga8")
            xn = x2        # aliases: x2[*,c] dead (stats read) before xn[*,c]
            G = A          # G written after A's last read each round

            for dt in range(DT):
                nc.vector.memset(Bv[:, dt * BVW: dt * BVW + MARG], 0.0)
                nc.vector.memset(Bvo[:, dt * BVW: dt * BVW + MARG + 1], 0.0)

            # round-1 input, chunked so stage-1 starts on the first chunk;
            # separate queue (gpsimd) so it overlaps the weight DMAs.
            # Later-stage weights follow on the same queue (sync stays free
            # for the Bvo shifted copies, which gate the first window adds).
            for (c0, cn) in CHUNKS:
                for dt in range(DT):
                    nc.gpsimd.dma_start(out=h0[:, dt * T + c0: dt * T + c0 + cn],
                                        in_=h_in[dt, :, c0: c0 + cn])
            for t_sb, t_d in ((u1t, u1t_d), (wf, wf_d), (u2, u2_d)):
                nc.gpsimd.dma_start(
                    out=t_sb[:].rearrange("p (k d) -> p k d", k=DT),
                    in_=t_d.rearrange("k p d -> p k d"))
            nc.gpsimd.dma_start(
                out=idg[:].rearrange("p (k q) -> p k q", k=DT),
                in_=idg_d.rearrange("k p q -> p k q"))

            # HAM warm-up: ~3.5us of dense dummy matmuls while the PE waits
            # for the first h chunk's DMA, so stage 1 starts at 2.4 GHz
            # (cold-start otherwise runs the whole first chunk at half clock).
            for i in range(36):
                pd = psr.tile([P, 512], f32, tag="prow")
                nc.tensor.matmul(pd[:, :128], ones_sq[:], ones_sq[:, :128],
                                 start=True, stop=True)

            def hsl(h, dt, c0, n):
                return h[:, dt * T + c0: dt * T + c0 + n]

            def wtile(w, kt, dt):
                return w[:, kt * D + dt * P: kt * D + dt * P + P]

            hbufs = [h0, h1]

            def emit_stats(hout, c0, cn):
                """LN-stats matmuls + moment prep for tokens [c0, c0+cn)."""
                pr0 = psr.tile([P, 512], f32, tag="prow")
                pr1 = psr.tile([P, 512], f32, tag="prow")
                for kt in range(DT):
                    nc.tensor.matmul(pr0[:, :cn], ones_sq[:],
                                     hsl(hout, kt, c0, cn),
                                     start=(kt == 0), stop=(kt == DT - 1))
                for kt in range(DT):
                    nc.tensor.matmul(pr1[:, :cn], ones_sq[:],
                                     x2[:, kt * T + c0: kt * T + c0 + cn],
                                     start=(kt == 0), stop=(kt == DT - 1))
                mu2 = wp.tile([P, CN], f32, tag="mu2")
                nc.vector.tensor_scalar_mul(nmub[:, c0: c0 + cn],
                                            pr0[:, :cn], -1.0 / D)
                nc.gpsimd.tensor_tensor(mu2[:, :cn], nmub[:, c0: c0 + cn],
                                        nmub[:, c0: c0 + cn], ALU.mult)
                nc.vector.scalar_tensor_tensor(
                    varb[:, c0: c0 + cn], pr1[:, :cn],
                    1.0 / D, mu2[:, :cn], ALU.mult, ALU.subtract)

            def emit_rsqrt(lo, hi, mask_halo):
                nc.scalar.activation(rstb[:, lo:hi], varb[:, lo:hi],
                                     AF.Abs_reciprocal_sqrt, bias=EPS)
                if mask_halo:
                    nc.vector.tensor_tensor(rstb[:, 0:HALO], rstb[:, 0:HALO],
                                            hmask[:], ALU.mult)

            def emit_apply(hout, c0, cn, eng, full_ln):
                """y = (new_h - mu) * rstd  (plus gamma/beta on the final
                round) for tokens [c0, c0+cn)."""
                rsl = rstb[:, c0: c0 + cn]
                nsl = nmub[:, c0: c0 + cn]
                for dt in range(DT):
                    xs = xn[:, dt * T + c0: dt * T + c0 + cn]
                    ho = hsl(hout, dt, c0, cn)
                    eng.tensor_tensor(xs, ho, nsl, ALU.add)
                    if not full_ln:
                        eng.tensor_tensor(ho, xs, rsl, ALU.mult)
                    else:
                        eng.tensor_tensor(xs, xs, rsl, ALU.mult)
                        eng.tensor_scalar(ho, xs, lng[:, dt: dt + 1],
                                          lnb[:, dt: dt + 1],
                                          ALU.mult, ALU.add)

            for r in range(N_ROUNDS):
                hin = hbufs[r % 2]
                hout = hbufs[(r + 1) % 2]
                last = (r == N_ROUNDS - 1)
                chs = CHUNKS
                nch = len(chs)

                def stage1(c0, cn):
                    # ---- stage 1: A / Bv matmuls for one chunk
                    for dt in range(DT):
                        pa = psab.tile([P, 512], f32, tag="pab")
                        for kt in range(DT):
                            nc.tensor.matmul(pa[:, :cn], wtile(w1t, kt, dt),
                                             hsl(hin, kt, c0, cn),
                                             start=(kt == 0), stop=(kt == DT - 1))
                        nc.scalar.activation(A[:, dt * T + c0: dt * T + c0 + cn],
                                             pa[:, :cn], AF.Copy)
                        pb = psab.tile([P, 512], f32, tag="pab")
                        for kt in range(DT):
                            nc.tensor.matmul(pb[:, :cn], wtile(w1b, kt, dt),
                                             hsl(hin, kt, c0, cn),
                                             start=(kt == 0), stop=(kt == DT - 1))
                        base = dt * BVW + MARG + c0
                        nc.scalar.activation(Bv[:, base: base + cn], pb[:, :cn],
                                             AF.Copy)
                        # +1-shifted copy for odd taps, off the compute engines
                        nc.sync.dma_start(
                            out=Bvo[:, base + 1: base + 1 + cn],
                            in_=Bv[:, base: base + cn])

                for ci, (c0, cn) in enumerate(chs):
                    stage1(c0, cn)
                    # ---- stage 2: windowed gelu-sum -> S (this chunk)
                    for dt in range(DT):
                        tmp = wp.tile([P, W9 * CN], bf16, tag="tmp")
                        g = wp.tile([P, W9 * CN], bf16, tag="g")
                        a_sl = A[:, dt * T + c0: dt * T + c0 + cn]
                        base = dt * BVW + MARG + c0
                        # even taps k=8,6,4,2,0 <- Bv[base-8 + 2j]
                        nc.vector.tensor_tensor(
                            _win_ap(tmp[:, 0:cn], 5, cn),
                            _win_ap(a_sl, 5, 0),
                            _win_ap(Bv[:, base - 8: base - 8 + cn], 5, 2),
                            ALU.add)
                        # odd taps k=7,5,3,1 <- Bvo[base-6 + 2j]
                        nc.vector.tensor_tensor(
                            _win_ap(tmp[:, 5 * cn: 6 * cn], 4, cn),
                            _win_ap(a_sl, 4, 0),
                            _win_ap(Bvo[:, base - 6: base - 6 + cn], 4, 2),
                            ALU.add)
                        nc.scalar.activation(g[:, : W9 * cn], tmp[:, : W9 * cn],
                                             AF.Gelu, bias=b1[:, dt: dt + 1])
                        # keep-warm: a tiny matmul gated on this gelu fires
                        # every ~3us through the window phase so the PE HAM
                        # clock gate never re-throttles to 1.2 GHz during
                        # the PE-idle stretch (it halves matmul throughput
                        # for ~3.4us after every >3.4us idle window).
                        pd = psr.tile([P, 512], f32, tag="prow")
                        nc.tensor.matmul(pd[:, :128], u2[:, :P],
                                         g[:, :128], start=True, stop=True)
                        te = nc.vector
                        te.tensor_tensor(tmp[:, 0: 4 * cn], g[:, 0: 4 * cn],
                                         g[:, 4 * cn: 8 * cn], ALU.add)
                        te.tensor_tensor(tmp[:, 0: 2 * cn], tmp[:, 0: 2 * cn],
                                         tmp[:, 2 * cn: 4 * cn], ALU.add)
                        te.tensor_tensor(tmp[:, 0: cn], tmp[:, 0: cn],
                                         tmp[:, cn: 2 * cn], ALU.add)
                        te.tensor_tensor(
                            S[:, dt * T + c0: dt * T + c0 + cn],
                            tmp[:, 0: cn], g[:, 8 * cn: 9 * cn], ALU.add)

                    # ---- edge fixup (chunk 0 only; no-op off sequence starts)
                    if ci == 0:
                        for dt in range(DT):
                            sle = S[:, dt * T + HALO: dt * T + HALO + W]
                            nc.scalar.activation(
                                ga8[:], A[:, dt * T + HALO: dt * T + HALO + W],
                                AF.Gelu, bias=b1[:, dt: dt + 1])
                            nc.vector.tensor_tensor(ga8[:], ga8[:], edge_a[:],
                                                    ALU.mult)
                            nc.vector.tensor_tensor(sle, sle, ga8[:], ALU.subtract)
                            nc.vector.tensor_tensor(sle, sle, edge_s[:], ALU.mult)

                    # ---- stage 3: U = u1t.T@h + wf.T@S ; G = gelu(U+ub1')
                    for dt in range(DT):
                        pu = ps.tile([P, 512], f32, tag="pmm")
                        for kt in range(DT):
                            nc.tensor.matmul(pu[:, :cn], wtile(u1t, kt, dt),
                                             hsl(hin, kt, c0, cn),
                                             start=(kt == 0), stop=False)
                        for kt in range(DT):
                            nc.tensor.matmul(pu[:, :cn], wtile(wf, kt, dt),
                                             S[:, kt * T + c0: kt * T + c0 + cn],
                                             start=False, stop=(kt == DT - 1))
                        nc.scalar.activation(G[:, dt * T + c0: dt * T + c0 + cn],
                                             pu[:, :cn], AF.Gelu,
                                             bias=ub1[:, dt: dt + 1])

                    # ---- stage 4: V = u2.T@G (+ gamma*h via diag matmul) ; x^2
                    for dt in range(DT):
                        pv = ps.tile([P, 512], f32, tag="pmm")
                        for kt in range(DT):
                            nc.tensor.matmul(pv[:, :cn], wtile(u2, kt, dt),
                                             G[:, kt * T + c0: kt * T + c0 + cn],
                                             start=(kt == 0), stop=False)
                        nc.tensor.matmul(pv[:, :cn],
                                         idg[:, dt * P: dt * P + P],
                                         hsl(hin, dt, c0, cn),
                                         start=False, stop=True)
                        nc.scalar.activation(hsl(hout, dt, c0, cn), pv[:, :cn],
                                             AF.Identity, bias=ub2[:, dt: dt + 1])
                        nc.gpsimd.tensor_tensor(
                            x2[:, dt * T + c0: dt * T + c0 + cn],
                            hsl(hout, dt, c0, cn), hsl(hout, dt, c0, cn),
                            ALU.mult)

                    # ---- stage 5: stats + LN scheduling
                    emit_stats(hout, c0, cn)
                    if last:
                        # final round, per chunk so LN + out-DMA pipeline.
                        # gamma/beta are applied on the host (per-channel
                        # affine on the full output), so this is the cheap
                        # 2-op form too; middle chunk on GpSimd so the tail
                        # chunks run on two engines in parallel.
                        emit_rsqrt(c0, c0 + cn, False)
                        emit_apply(hout, c0, cn,
                                   nc.gpsimd if ci == 1 else nc.vector, False)
                    elif ci == nch - 2:
                        # rsqrt for chunks 0..1 in one ACT call; chunk 0's
                        # apply on Vector now (critical path to next round);
                        # chunk 1's apply is emitted after chunk 2's x^2 so
                        # the GpSimd queue can't stall chunk 2's stats.
                        emit_rsqrt(0, c0 + cn, True)
                        emit_apply(hout, chs[0][0], chs[0][1],
                                   nc.vector, False)
                    elif ci == nch - 1:
                        pc0, pcn = chs[nch - 2]
                        emit_apply(hout, pc0, pcn, nc.gpsimd, False)
                        emit_rsqrt(c0, c0 + cn, False)
                        emit_apply(hout, c0, cn, nc.gpsimd, False)

            hfin = hbufs[N_ROUNDS % 2]
            qs = [nc.sync, nc.gpsimd, nc.scalar]
            for dt in range(DT):
                for ci, (c0, cn) in enumerate(CHUNKS):
                    lo = max(c0, HALO)
                    hi = c0 + cn
                    qs[(dt + ci) % 3].dma_start(
                        out=out_d[dt, :, lo - HALO: hi - HALO],
                        in_=hsl(hfin, dt, lo, hi - lo).bitcast(f32))

    nc.finalize()
    return nc


_NC_CACHE = {}


def _get_nc():
    if "nc" not in _NC_CACHE:
        _NC_CACHE["nc"] = build_nc()
    return _NC_CACHE["nc"]


def _prep_inputs(chunk_summaries, msg_w1, msg_b1, msg_w2, msg_b2,
                 upd_w1, upd_b1, upd_w2, upd_b2, ln_g, ln_b):
    h = np.asarray(chunk_summaries, np.float32)          # (B, N, D)
    w1 = np.asarray(msg_w1, np.float32)                  # (2D, D)
    w2 = np.asarray(msg_w2, np.float32)                  # (D, D)
    u1 = np.asarray(upd_w1, np.float32)
    u2 = np.asarray(upd_w2, np.float32)
    b2 = np.asarray(msg_b2, np.float32)
    b1v = np.asarray(msg_b1, np.float32)
    gv = np.asarray(ln_g, np.float32)
    lnbv = np.asarray(ln_b, np.float32)

    # fuse the agg matmul: agg = (S/9)@w2 + b2 only feeds [h, agg] @ u1,
    # so U = h@u1[:D] + S@((w2/9)@u1[D:]) + (ub1 + b2@u1[D:]).
    # LN gamma/beta fold into the h-consuming weights/biases: the h buffers
    # carry y with h_true = y*gamma + beta, so scale w rows by gamma and
    # fold beta@w into biases; the residual uses a diag(gamma) matmul.
    wf = (w2 / 9.0) @ u1[D:]
    b1_fold = b1v + lnbv @ w1[:D] + lnbv @ w1[D:]
    ub1_fold = np.asarray(upd_b1, np.float32) + b2 @ u1[D:] + lnbv @ u1[:D]
    ub2_fold = np.asarray(upd_b2, np.float32) + lnbv
    w1t_s = w1[:D] * gv[:, None]
    w1b_s = w1[D:] * gv[:, None]
    u1t_s = u1[:D] * gv[:, None]
    idg = np.zeros((DT, P, P), np.float32)
    for dt in range(DT):
        np.fill_diagonal(idg[dt], gv[dt * P:(dt + 1) * P])

    def pack_w(w, dt_np):
        return np.ascontiguousarray(w.reshape(DT, P, D).astype(dt_np))

    def pack_b2(b):
        return np.ascontiguousarray(np.asarray(b, np.float32).reshape(DT, P).T)

    common = {
        "w1t": pack_w(w1t_s, np.float32),
        "w1b": pack_w(w1b_s, np.float32),
        "u1t": pack_w(u1t_s, np.float32),
        "wf": pack_w(wf, ml_dtypes.bfloat16),
        "u2": pack_w(u2, ml_dtypes.bfloat16),
        "b1": pack_b2(b1_fold),
        "ub1": pack_b2(ub1_fold),
        "ub2": pack_b2(ub2_fold),
        "lng": pack_b2(ln_g),
        "lnb": pack_b2(ln_b),
        "idg": idg,
    }

    i8 = np.arange(W, dtype=np.float32)
    ea_edge = np.broadcast_to((W - i8), (P, W)).astype(ml_dtypes.bfloat16)
    es_edge = np.broadcast_to((9.0 / (i8 + 1.0)), (P, W)).astype(ml_dtypes.bfloat16)
    ea_mid = np.zeros((P, W), ml_dtypes.bfloat16)
    es_mid = np.ones((P, W), ml_dtypes.bfloat16)
    hm_edge = np.zeros((P, HALO), np.float32)
    hm_mid = np.ones((P, HALO), np.float32)

    # round-0 input in the y convention: y0 = (h - beta) / gamma (0 if g=0;
    # those channels' true contribution beta@w already rides the biases)
    ginv = np.where(gv != 0.0, 1.0 / np.where(gv == 0.0, 1.0, gv), 0.0)

    in_maps = []
    for core in range(NCORES):
        b = core // 4
        q = core % 4
        n0 = q * NLOC
        if q == 0:
            loc = np.zeros((T, D), np.float32)
            loc[HALO:] = (h[b, :NLOC] - lnbv) * ginv
            ea, es, hm = ea_edge, es_edge, hm_edge
        else:
            loc = (h[b, n0 - HALO: n0 + NLOC] - lnbv) * ginv
            ea, es, hm = ea_mid, es_mid, hm_mid
        hloc = np.ascontiguousarray(loc.T.reshape(DT, P, T))
        m = dict(common)
        m["h_in"] = hloc
        m["edge_a"] = ea
        m["edge_s"] = es
        m["hmask"] = hm
        in_maps.append(m)
    return in_maps


def kernel(**inputs) -> np.ndarray:
    nc = _get_nc()
    in_maps = _prep_inputs(**inputs)
    res = run_bass_kernel_spmd(nc, in_maps, list(range(NCORES)))
    out = np.empty((B, N, D), np.float32)
    for core in range(NCORES):
        b = core // 4
        q = core % 4
        o = res.results[core]["out"]          # (DT, P, NLOC)
        out[b, q * NLOC:(q + 1) * NLOC] = o.reshape(D, NLOC).T
    # final LN gamma/beta applied host-side (device emits y = (x-mu)*rstd)
    out *= np.asarray(inputs["ln_g"], np.float32)
    out += np.asarray(inputs["ln_b"], np.float32)
    return out
